# revision 1
# baseline (speedup 1.0000x reference)
"""Multi-head attention (B=4, T=S=2048, E=1024, H=16, D=64) on 8 TRN2 NeuronCores.

Sharding: core c handles batch b=c//2 and head-group g=c%2 (8 of 16 heads).
Each core computes its 8 heads' attention plus the matching column-slice of
the output projection, producing a partial [T, E] f32 output. Host sums the
two partials per batch and adds bo.

On-chip dataflow (all matmuls bf16 with fp32 PSUM accumulation):
  qT[d,t] = WqT.T @ queryT       (d-major projections, per 128-dim head pair)
  kT[d,t] likewise; v[s,d] natural via value.T as the stationary operand
  S.T[s,t] = kT_h.T @ qT_h       (two heads row-packed in the 128-row PE array)
  expS.T   = exp(S.T * 1/8)      (ScalarE, PSUM -> SBUF bf16)
  [O.T;den]= [v_h|1].T @ expS.T  (ones-augmented stationary -> denominators)
  Onorm    = O.T * (1/den)       (reciprocal + GPSIMD partition broadcast)
  partial  = Onorm.T @ WoSlice   (accumulate over the core's 4 head pairs)

Emission is software-pipelined: stage s=(pair, t-quarter) in pair-major
order; each stage's 16 score-tile slots interleave the previous stage's PV
accumulation plus spread-out projection / v-projection / out-projection
work, keeping ScalarE (the exp bottleneck) continuously fed.
"""

from contextlib import ExitStack

import numpy as np
import ml_dtypes

B, T, S, E = 4, 2048, 2048, 1024
H, D = 16, 64
DC = 512          # dims per core (8 heads x 64)
NP = 4            # head pairs per core
NS = S // 128     # 16 s-tiles
NQ = 4            # t-quarters of 512

_BF16 = ml_dtypes.bfloat16

_cached = None


def _build(repeats=1):
    import concourse.bass as bass
    import concourse.mybir as mybir
    import concourse.tile as tile
    from concourse import bacc

    f32 = mybir.dt.float32
    bf16 = mybir.dt.bfloat16
    AF = mybir.ActivationFunctionType

    nc = bacc.Bacc("TRN2", target_bir_lowering=False)

    qT_d = nc.dram_tensor("qT", [E, T], bf16, kind="ExternalInput")
    kT_d = nc.dram_tensor("kT", [E, S], bf16, kind="ExternalInput")
    vT_d = nc.dram_tensor("vT", [E, S], bf16, kind="ExternalInput")
    WqT_d = nc.dram_tensor("WqT", [E, DC], bf16, kind="ExternalInput")
    WkT_d = nc.dram_tensor("WkT", [E, DC], bf16, kind="ExternalInput")
    WvT_d = nc.dram_tensor("WvT", [E, DC], bf16, kind="ExternalInput")
    WoS_d = nc.dram_tensor("WoS", [DC, E], bf16, kind="ExternalInput")
    bq_d = nc.dram_tensor("bq", [128, NP], f32, kind="ExternalInput")
    bk_d = nc.dram_tensor("bk", [128, NP], f32, kind="ExternalInput")
    bv_d = nc.dram_tensor("bv", [1, DC], f32, kind="ExternalInput")
    out_d = nc.dram_tensor("out", [T, E], f32, kind="ExternalOutput")

    with tile.TileContext(nc) as tc, ExitStack() as ctx:
        persist = ctx.enter_context(tc.tile_pool(name="persist", bufs=1))
        psc = ctx.enter_context(tc.tile_pool(name="psc", bufs=2, space="PSUM"))
        ppv = ctx.enter_context(tc.tile_pool(name="ppv", bufs=2, space="PSUM"))
        pmx = ctx.enter_context(tc.tile_pool(name="pmx", bufs=2, space="PSUM"))
        expool = ctx.enter_context(tc.tile_pool(name="expool", bufs=22))
        small = ctx.enter_context(tc.tile_pool(name="small", bufs=3))
        ocp_pool = ctx.enter_context(tc.tile_pool(name="ocp", bufs=3))
        xin = ctx.enter_context(tc.tile_pool(name="xin", bufs=14))
        wpool = ctx.enter_context(tc.tile_pool(name="wts", bufs=24))

        # ---- persistent SBUF tiles ----
        qTs = [persist.tile([128, T], bf16, tag=f"qT{p}", name=f"qT{p}") for p in range(NP)]
        kTs = [persist.tile([128, S], bf16, tag=f"kT{p}", name=f"kT{p}") for p in range(NP)]
        vaug = [persist.tile([128, 8 * 65], bf16, tag=f"va{st}", name=f"va{st}") for st in range(NS)]
        WoSs = [persist.tile([128, E], bf16, tag=f"wo{p}", name=f"wo{p}") for p in range(NP)]
        Onorm = [persist.tile([128, T], bf16, tag=f"on{p}", name=f"on{p}") for p in range(NP)]
        bq_sb = persist.tile([128, NP], f32, tag="bq", name="bq_sb")
        bk_sb = persist.tile([128, NP], f32, tag="bk", name="bk_sb")
        bv_sb = persist.tile([128, DC], f32, tag="bv", name="bv_sb")

        nc.sync.dma_start(out=bq_sb, in_=bq_d[:, :])
        nc.sync.dma_start(out=bk_sb, in_=bk_d[:, :])
        bv_ap = bv_d[:, :]
        bv_bcast_ap = bass.AP(
            tensor=bv_ap.tensor,
            offset=bv_ap.offset,
            ap=[[0, 128], bv_ap.ap[-1]],
        )
        nc.sync.dma_start(out=bv_sb, in_=bv_bcast_ap)
        for p in range(NP):
            nc.sync.dma_start(out=WoSs[p], in_=WoS_d[p * 128:(p + 1) * 128, :])
        for st in range(NS):
            va3 = vaug[st].rearrange("p (h x) -> p h x", x=65)
            nc.vector.memset(va3[:, :, 64:65], 1.0)

        def load_wtiles(dram):
            ts_ = []
            for e in range(8):
                t_ = wpool.tile([128, DC], bf16, tag="w", name="wt")
                nc.sync.dma_start(out=t_, in_=dram[e * 128:(e + 1) * 128, :])
                ts_.append(t_)
            return ts_

        def proj_thunks(p, x_dram, w_tiles, dst, bias_sb, halves=(0, 1),
                        preload=False):
            """One pair's q/k projection as a thunk list: two column-halves;
            per half, stream 8 e-tile chunks (load + 2 quarter-MMs each),
            then bias-add the two finished quarters out of PSUM. With
            preload=True all 8 DMAs are issued before the first matmul
            (used for the serial startup blocks)."""
            thunks = []
            for half in halves:
                ps_pair = []  # the two quarter psums of this half (alloc lazily)
                xh = []

                def open_half(half=half, ps_pair=ps_pair, xh=xh):
                    for qi in range(2):
                        ps_pair.append(pmx.tile([128, 512], f32, tag="mx", name="mx_ps"))
                    if preload:
                        for e in range(8):
                            xt = xin.tile([128, 1024], bf16, tag="xin", name="xin")
                            nc.sync.dma_start(
                                out=xt,
                                in_=x_dram[e * 128:(e + 1) * 128,
                                           half * 1024:(half + 1) * 1024])
                            xh.append(xt)

                thunks.append(open_half)

                for e in range(8):
                    def echunk(e=e, half=half, ps_pair=ps_pair, xh=xh):
                        if preload:
                            xt = xh[e]
                        else:
                            xt = xin.tile([128, 1024], bf16, tag="xin", name="xin")
                            nc.sync.dma_start(
                                out=xt,
                                in_=x_dram[e * 128:(e + 1) * 128,
                                           half * 1024:(half + 1) * 1024])
                        for qi in range(2):
                            nc.tensor.matmul(
                                ps_pair[qi],
                                w_tiles[e][:, p * 128:(p + 1) * 128],
                                xt[:, qi * 512:(qi + 1) * 512],
                                start=(e == 0),
                                stop=(e == 7),
                            )
                    thunks.append(echunk)

                def close_half(half=half, ps_pair=ps_pair):
                    for qi in range(2):
                        q = half * 2 + qi
                        nc.vector.tensor_scalar_add(
                            dst[:, q * 512:(q + 1) * 512],
                            ps_pair[qi], bias_sb[:, p:p + 1])
                    ps_pair.clear()

                thunks.append(close_half)
            return thunks

        def vproj_thunks(wv_tiles, dh):
            """V projection for head-quad dh (4 heads, N=256), streamed in
            two s-halves. dh=0 feeds pairs 0-1 (needed by stage 1); dh=1
            feeds pairs 2-3 (needed from stage 9) and can spread late."""
            thunks = []
            for half in range(2):
                vh = []

                def load_half(half=half, vh=vh):
                    for e in range(8):
                        vt = xin.tile([128, 1024], bf16, tag="xin", name="xin")
                        nc.sync.dma_start(
                            out=vt,
                            in_=vT_d[e * 128:(e + 1) * 128,
                                     half * 1024:(half + 1) * 1024])
                        vh.append(vt)

                thunks.append(load_half)
                for sti in range(8):
                    def vst(sti=sti, half=half, vh=vh):
                        st = half * 8 + sti
                        # stages with live PV accumulators own the "pv"
                        # slots -> everything except (dh0, s-half0) uses "mx"
                        if dh == 0 and half == 0:
                            ps = ppv.tile([128, 512], f32, tag="pv", name="pv_ps")
                        else:
                            ps = pmx.tile([128, 512], f32, tag="mx", name="mx_ps")
                        for e in range(8):
                            nc.tensor.matmul(
                                ps[:, 0:256],
                                vh[e][:, sti * 128:(sti + 1) * 128],
                                wv_tiles[e][:, dh * 256:(dh + 1) * 256],
                                start=(e == 0),
                                stop=(e == 7),
                            )
                        va3 = vaug[st].rearrange("p (h x) -> p h x", x=65)
                        nc.vector.tensor_add(
                            va3[:, dh * 4:(dh + 1) * 4, 0:64],
                            ps[:, 0:256].rearrange("p (h x) -> p h x", x=64),
                            bv_sb[:, dh * 256:(dh + 1) * 256].rearrange(
                                "p (h x) -> p h x", x=64),
                        )
                        if half == 0 and sti == 7:
                            vh.clear()
                    thunks.append(vst)
            return thunks

        def outproj_thunks(tq):
            thunks = []
            for tt in range(tq * 4, tq * 4 + 4):
                for c in range(2):
                    def unit(tt=tt, c=c):
                        op_ps = pmx.tile([128, 512], f32, tag="mx", name="mx_ps")
                        for p in range(NP):
                            nc.tensor.matmul(
                                op_ps,
                                Onorm[p][:, tt * 128:(tt + 1) * 128],
                                WoSs[p][:, c * 512:(c + 1) * 512],
                                start=(p == 0),
                                stop=(p == 3),
                            )
                        oc = ocp_pool.tile([128, 512], f32, tag="ocp", name="oc")
                        nc.vector.tensor_copy(oc, op_ps)
                        nc.sync.dma_start(
                            out=out_d[tt * 128:(tt + 1) * 128,
                                      c * 512:(c + 1) * 512],
                            in_=oc)
                    thunks.append(unit)
            return thunks

        def outproj_tail(tq):
            # tail variant: "sc" psum tiles are free once scoring has ended,
            # so use wide [128,1024] units to avoid mx-slot serialization
            for tt in range(tq * 4, tq * 4 + 4):
                op_ps = psc.tile([128, 1024], f32, tag="sc", name="sc_ps")
                for c in range(2):
                    for p in range(NP):
                        nc.tensor.matmul(
                            op_ps[:, c * 512:(c + 1) * 512],
                            Onorm[p][:, tt * 128:(tt + 1) * 128],
                            WoSs[p][:, c * 512:(c + 1) * 512],
                            start=(p == 0),
                            stop=(p == 3),
                        )
                oc = ocp_pool.tile([128, 1024], f32, tag="ocpw", name="ocw")
                nc.vector.tensor_copy(oc, op_ps)
                nc.sync.dma_start(out=out_d[tt * 128:(tt + 1) * 128, :], in_=oc)

        class PrevStage:
            def __init__(self, p, tq, exs):
                self.p, self.tq, self.exs = p, tq, exs
                self.o_ps = None

        def emit_pv_mm(prev, h, st):
            if prev.o_ps is None:
                prev.o_ps = [None, None]
            if prev.o_ps[h] is None:
                prev.o_ps[h] = ppv.tile([128, 512], f32, tag="pv", name="pv_ps")
            hidx = 2 * prev.p + h
            nc.tensor.matmul(
                prev.o_ps[h][0:65, :],
                vaug[st][:, hidx * 65:hidx * 65 + 65],
                prev.exs[st][:, h * 512:(h + 1) * 512],
                start=(st == 0),
                stop=(st == 15),
            )

        def emit_pv_slot(prev, st):
            emit_pv_mm(prev, 0, st)
            emit_pv_mm(prev, 1, st)

        def emit_normalize(prev):
            t0 = prev.tq * 512
            for h in range(2):
                o_ps = prev.o_ps[h]
                # stage O out of PSUM immediately so the pv slots free after
                # two short DVE ops instead of the whole bcast chain; the
                # final multiply then runs in DVE 4x bf16 mode (all SBUF).
                rc = small.tile([1, 512], f32, tag="rc", name="rc")
                nc.vector.reciprocal(rc, o_ps[64:65, :])
                ocp = small.tile([64, 512], f32, tag="oc2", name="oc2")
                nc.vector.tensor_copy(ocp, o_ps[0:64, :])
                rcb = small.tile([1, 512], bf16, tag="rcb", name="rcb")
                nc.vector.tensor_copy(rcb, rc)
                rb_sb = small.tile([64, 512], bf16, tag="rb", name="rb")
                nc.gpsimd.partition_broadcast(rb_sb, rcb[0:1, :])
                nc.vector.tensor_mul(
                    Onorm[prev.p][h * 64:(h + 1) * 64, t0:t0 + 512],
                    ocp,
                    rb_sb,
                )

        def emit_stage(p, tq, prev, extras, dl=6):
            """16 score slots for (p, tq); interleave prev stage's PV and
            the extra thunks (all emitted by slot `dl`); returns this
            stage's PrevStage record."""
            t0 = tq * 512
            exs = []
            n_ex = len(extras)
            taken = 0
            for st in range(NS):
                sc_ps = psc.tile([128, 1024], f32, tag="sc", name="sc_ps")
                nc.tensor.matmul(
                    sc_ps[:, 0:512],
                    kTs[p][0:64, st * 128:(st + 1) * 128],
                    qTs[p][0:64, t0:t0 + 512],
                    start=True, stop=True,
                    tile_position=(0, 0),
                )
                nc.tensor.matmul(
                    sc_ps[:, 512:1024],
                    kTs[p][64:128, st * 128:(st + 1) * 128],
                    qTs[p][64:128, t0:t0 + 512],
                    start=True, stop=True,
                    tile_position=(64, 0),
                )
                ex = expool.tile([128, 1024], bf16, tag="ex", name="ex")
                nc.scalar.activation(ex, sc_ps, AF.Exp, scale=0.125)
                exs.append(ex)
                if prev is not None:
                    emit_pv_slot(prev, st)
                want = (n_ex * min(st + 1, dl) + dl - 1) // dl
                while taken < want:
                    extras[taken]()
                    taken += 1
            while taken < n_ex:
                extras[taken]()
                taken += 1
            if prev is not None:
                emit_normalize(prev)
            return PrevStage(p, tq, exs)

        # ---- emission ----
        for _rep in range(repeats):
            # startup: only the first column-halves of pair-0's q/k
            # projections block the first scores; everything else overlaps.
            wq_tiles = load_wtiles(WqT_d)
            q0h0 = proj_thunks(0, qT_d, wq_tiles, qTs[0], bq_sb,
                               halves=(0,), preload=True)
            wk_tiles = load_wtiles(WkT_d)
            k0h0 = proj_thunks(0, kT_d, wk_tiles, kTs[0], bk_sb,
                               halves=(0,), preload=True)
            # issue both halves' preload DMAs before any matmul runs
            q0h0[0](); k0h0[0]()
            for th in q0h0[1:]:
                th()
            for th in k0h0[1:]:
                th()
            q0 = proj_thunks(0, qT_d, wq_tiles, qTs[0], bq_sb, halves=(1,))
            k0 = proj_thunks(0, kT_d, wk_tiles, kTs[0], bk_sb, halves=(1,))
            wv_tiles = load_wtiles(WvT_d)

            # per-stage extra work, placed just-in-time:
            #  stage 0: K0/Q0 second halves + V-projection first s-half
            #  stage 1: V-projection second s-half
            #  pair p>=1: K-half0 @4p-2, Q-half0 @4p-1, K-half1 @4p, Q-half1 @4p+1
            #  stages 14, 15: out-proj for t0, t1
            vpA = vproj_thunks(wv_tiles, 0)
            extras = {0: k0 + q0 + vpA[:10], 1: vpA[10:]}
            for p in range(1, NP):
                qp = proj_thunks(p, qT_d, wq_tiles, qTs[p], bq_sb)
                kp = proj_thunks(p, kT_d, wk_tiles, kTs[p], bk_sb)
                for sg, th in ((4 * p - 2, kp[:10]), (4 * p - 1, qp[:10]),
                               (4 * p, kp[10:]), (4 * p + 1, qp[10:])):
                    extras[sg] = extras.get(sg, []) + th
            # second head-quad of V, appended after each stage's proj work
            # (sequential mx-slot handoff, done well before stage 9 needs
            # it). Skip stages 4/5 whose proj work has a hard slot-8
            # deadline (dl=7) — vpB has no deadline and shouldn't compete.
            vpB = vproj_thunks(wv_tiles, 1)
            n4 = (len(vpB) + 3) // 4
            for i, sg in enumerate((2, 3, 6, 7)):
                extras[sg] = extras.get(sg, []) + vpB[i * n4:(i + 1) * n4]
            extras[14] = extras.get(14, []) + outproj_thunks(0)
            extras[15] = extras.get(15, []) + outproj_thunks(1)

            # pacing deadlines: K-half1 stages (4p) must finish extras by
            # slot 8 (their own scores need those kT columns); stages 0/1
            # feed vaug just-in-time; elsewhere spread smoothly.
            dls = {0: 14, 1: 14, 4: 7, 8: 7, 12: 7}
            prev = None
            for s in range(16):
                p, tq = s // 4, s % 4
                prev = emit_stage(p, tq, prev, extras.get(s, []),
                                  dl=dls.get(s, 16))

            # tail: PV of the last stage with out-proj(t2) interleaved
            # (its Onorm slices completed at the end of stage 15), then the
            # final normalize and out-proj(t3)
            op2 = outproj_thunks(2)
            for st in range(NS):
                emit_pv_slot(prev, st)
                if st % 2 == 1:
                    op2[st // 2]()
            emit_normalize(prev)
            outproj_tail(3)

    nc.compile()
    return nc


def _get_nc():
    global _cached
    if _cached is None:
        _cached = _build()
    return _cached


def _prep_core_inputs(c, query, key, value, Wq, Wk, Wv, Wo, bq, bk, bv,
                      _cache={}):
    b, g = c // 2, c % 2
    sl = slice(g * DC, (g + 1) * DC)
    key_ = (id(query), b)
    if key_ not in _cache:
        # both cores of a batch share the transposed/cast activations
        _cache.clear()
        _cache[key_] = {
            "qT": query[b].T.astype(_BF16),
            "kT": key[b].T.astype(_BF16),
            "vT": value[b].T.astype(_BF16),
        }
    shared = _cache[key_]
    return {
        **shared,
        "WqT": Wq[sl].T.astype(_BF16),
        "WkT": Wk[sl].T.astype(_BF16),
        "WvT": Wv[sl].T.astype(_BF16),
        "WoS": Wo[:, sl].T.astype(_BF16),
        "bq": np.ascontiguousarray(bq[sl].reshape(NP, 128).T),
        "bk": np.ascontiguousarray(bk[sl].reshape(NP, 128).T),
        "bv": np.ascontiguousarray(bv[sl].reshape(1, DC)),
    }


def kernel(**inputs):
    from concourse.bass_utils import run_bass_kernel_spmd

    args = {k: np.asarray(inputs[k], np.float32)
            for k in ("query", "key", "value", "Wq", "Wk", "Wv", "Wo",
                      "bq", "bk", "bv", "bo")}
    _prep_core_inputs.__defaults__[0].clear()
    nc = _get_nc()
    in_maps = [
        _prep_core_inputs(c, args["query"], args["key"], args["value"],
                          args["Wq"], args["Wk"], args["Wv"], args["Wo"],
                          args["bq"], args["bk"], args["bv"])
        for c in range(8)
    ]
    res = run_bass_kernel_spmd(nc, in_maps, core_ids=list(range(8)))
    outs = [r["out"] for r in res.results]
    final = np.empty((B, T, E), np.float32)
    for b in range(B):
        final[b] = outs[2 * b] + outs[2 * b + 1] + args["bo"][None, :]
    return final

